# revision 12
# baseline (speedup 1.0000x reference)
"""Trainium2 Bass kernel for nn_DepthwiseCrossViTMAE (criss-cross multihead self-attention).

Reference computation per token t (B*L = 4096 tokens, hidden 2048 = C*K with C=32, K=64):
  qkv[c, :] = x[t, c*64:(c+1)*64] @ Wqkv[c] + bqkv[c]          (per-channel linear)
  q, k, v = split(qkv)                                          each (C, K)
  for each d in [0, 64):  S_d = outer(q[:, d], k[:, d]) / 8     (C x C)
                          A_d = softmax_rows(S_d)
                          ctx[d, m] = sum_c A_d[c, m] * v[c, d]
  out[t, c*64:(c+1)*64] = ctx.T[c] @ Wout + bout

Sharding: data-parallel over the 4096 tokens, 512 tokens per core on 8 cores.

Per-core layout: SBUF partitions p = (t2, d) with t2 in {0,1}, d in [0,64);
token t = t2*256 + tau.  q/k/v live as [p, (tau, c)] bf16; k is pair-doubled
([tau, e, 2]) so the score outer-product hits the DVE bf16 2x mode.  Scores
per chunk of Tc taus: [p, (tau, e, c)] (c innermost):
  s[p,tau,e,c] = q[p,tau,c] * k[p,tau,e]     (DVE 2x via pair-doubled k)
  E = exp(s)                                  (ACT)
  z[p,tau,c] = sum_e E        (PE: 32 accumulating identity matmuls -> PSUM)
  w = v * 1/z                                 (DVE, small)
  P[p,tau,e,c] = E * w[bcast e]               (DVE 2x)
  ctx[p,tau,e] = sum_c P      (PE: 32 accumulating identity matmuls -> PSUM)
PE also does the QKV projections (bias folded in via rank-1 ones matmuls),
x transposes, and the output projection.  DVE and GPSIMD share SBUF ports
(concurrent use starves DVE), so GPSIMD is left idle.
"""

import sys

sys.path.insert(0, "/opt/trn_rl_repo")

import numpy as np
from contextlib import ExitStack

import concourse.bass as bass
import concourse.bacc as bacc
import concourse.mybir as mybir
import concourse.tile as tile
from concourse.masks import make_identity

F32 = mybir.dt.float32
BF16 = mybir.dt.bfloat16
AF = mybir.ActivationFunctionType
ALU = mybir.AluOpType
AX = mybir.AxisListType

C = 32          # channels
K = 64          # per-channel width (also Wout dim)
F = C * K       # hidden = 2048
N_CORES = 8


def build_nc(T=512, Tc=8, flush=8):
    """Build the single-core Bass module for T tokens.

    Tc: tau-chunk size for the attention loop (free size per elementwise op
    = Tc*1024).
    flush: chunks per output-DMA flush group.
    """
    T2 = 2
    TH = T // T2            # tokens per half (tau range)
    NFC = F // 128          # 16 feature chunks of x
    NTT = (T + 127) // 128  # token tiles of x
    NCH = TH // Tc          # attention chunks
    NSUB = Tc // 4          # out-proj sub-blocks (4 taus x 32 m = 128 rows)
    CG = 8                  # channels per QKV PSUM group
    assert TH % Tc == 0 and NCH % flush == 0 and Tc % 4 == 0

    nc = bacc.Bacc()
    x_d = nc.dram_tensor("x", [T, F], BF16, kind="ExternalInput")
    wq_d = nc.dram_tensor("wq", [F, K], BF16, kind="ExternalInput")
    wk_d = nc.dram_tensor("wk", [F, K], BF16, kind="ExternalInput")
    wv_d = nc.dram_tensor("wv", [F, K], BF16, kind="ExternalInput")
    # per-channel bias rows, laid out [C, 128] over the (t2,d) partition dim
    bq_d = nc.dram_tensor("bq", [C, 128], BF16, kind="ExternalInput")
    bk_d = nc.dram_tensor("bk", [C, 128], BF16, kind="ExternalInput")
    bv_d = nc.dram_tensor("bv", [C, 128], BF16, kind="ExternalInput")
    wo_d = nc.dram_tensor("wout", [128, K], BF16, kind="ExternalInput")
    bo_d = nc.dram_tensor("bout", [128, K], BF16, kind="ExternalInput")
    out_d = nc.dram_tensor("out", [T, F], F32, kind="ExternalOutput")

    with ExitStack() as octx, nc.allow_low_precision(reason="bf16 attention"):
        tc = octx.enter_context(tile.TileContext(nc))
        const_pool = octx.enter_context(tc.tile_pool(name="const", bufs=1))
        qkv_pool = octx.enter_context(tc.tile_pool(name="qkv", bufs=1))

        ident = const_pool.tile([128, 128], BF16)
        make_identity(nc, ident[:])
        ones_row = const_pool.tile([1, 128], BF16)
        nc.gpsimd.memset(ones_row[:], 1.0)

        bqT_sb = const_pool.tile([1, C, 128], BF16)
        bkT_sb = const_pool.tile([1, C, 128], BF16)
        bvT_sb = const_pool.tile([1, C, 128], BF16)
        wo_sb = const_pool.tile([128, K], BF16)
        bo_sb = const_pool.tile([128, K], BF16)
        nc.sync.dma_start(bqT_sb[:], bq_d[:].rearrange("c p -> (c p)").unsqueeze(0))
        nc.sync.dma_start(bkT_sb[:], bk_d[:].rearrange("c p -> (c p)").unsqueeze(0))
        nc.sync.dma_start(bvT_sb[:], bv_d[:].rearrange("c p -> (c p)").unsqueeze(0))
        nc.sync.dma_start(wo_sb[:], wo_d[:])
        nc.sync.dma_start(bo_sb[:], bo_d[:])

        # q is pre-scaled by 1/sqrt(K) on the host (weights and bias).
        qs_sb = qkv_pool.tile([128, TH, C], BF16)
        k_sb = qkv_pool.tile([128, TH, C, 2], BF16)
        v_sb = qkv_pool.tile([128, TH, C], BF16)

        # ---------------- phase 1: load x, transpose, QKV projections ----
        with (
            tc.tile_pool(name="xload", bufs=2) as xpool,
            tc.tile_pool(name="xt", bufs=1) as xtpool,
            tc.tile_pool(name="wgt", bufs=1) as wpool,
            tc.tile_pool(name="ps_qkv", bufs=3, space="PSUM") as ps1,
            tc.tile_pool(name="ps_tr", bufs=2, space="PSUM") as pst,
        ):
            wq_sb = wpool.tile([128, NFC, K], BF16)
            wk_sb = wpool.tile([128, NFC, K], BF16)
            wv_sb = wpool.tile([128, NFC, K], BF16)
            nc.sync.dma_start(wq_sb[:], wq_d[:].rearrange("(fc p) d -> p fc d", p=128))
            nc.sync.dma_start(wk_sb[:], wk_d[:].rearrange("(fc p) d -> p fc d", p=128))
            nc.sync.dma_start(wv_sb[:], wv_d[:].rearrange("(fc p) d -> p fc d", p=128))

            # xT[feat, token] per 128-feature chunk, via PE transpose.
            # Token-tile order 0,2,1,3 puts both t2-halves of the earliest
            # taus first so the attention loop can start sooner.
            xt_sb = xtpool.tile([128, NFC, T], BF16)
            tt_order = [0, 2, 1, 3] if NTT == 4 else list(range(NTT))
            for tt in tt_order:
                trows = min(128, T - tt * 128)
                x_sb = xpool.tile([128, F], BF16)
                for fc in range(NFC):
                    nc.sync.dma_start(
                        x_sb[:trows, fc * 128 : (fc + 1) * 128],
                        x_d[tt * 128 : tt * 128 + trows, fc * 128 : (fc + 1) * 128],
                    )
                for fc in range(NFC):
                    ps_t = pst.tile([128, 128], BF16)
                    nc.tensor.transpose(
                        ps_t[:, :trows],
                        x_sb[:trows, fc * 128 : (fc + 1) * 128],
                        ident[:trows, :trows],
                    )
                    nc.vector.tensor_copy(
                        xt_sb[:, fc, tt * 128 : tt * 128 + trows], ps_t[:, :trows]
                    )

            # per-channel QKV projections in groups of CG channels per PSUM
            # tile; bias folded in via a rank-1 ones matmul; one batched DVE
            # copy moves each group PSUM -> SBUF (bf16, c-interleaved).
            NQ = 2
            HH = TH // NQ
            for half in range(NQ):
                hsl = slice(half * HH, half * HH + HH)
                for qkv_i, (w_sb, bT_sb) in enumerate(
                    (
                        (wq_sb, bqT_sb),
                        (wk_sb, bkT_sb),
                        (wv_sb, bvT_sb),
                    )
                ):
                    for cg in range(C // CG):
                        ps = ps1.tile([128, CG, HH], F32)
                        for ci in range(CG):
                            c = cg * CG + ci
                            fc, h = divmod(c, 2)
                            hp = slice(64 * h, 64 * h + 64)
                            for t2 in range(T2):
                                nc.tensor.matmul(
                                    ps[64 * t2 : 64 * t2 + 64, ci, :],
                                    w_sb[hp, fc, :],
                                    xt_sb[
                                        hp,
                                        fc,
                                        t2 * TH + half * HH : t2 * TH + half * HH + HH,
                                    ],
                                    start=True,
                                    stop=False,
                                )
                            # bias: ones[1,HH] broadcast against bT[1,128]
                            nc.tensor.matmul(
                                ps[:, ci, :],
                                bT_sb[0:1, c, :],
                                ones_row[0:1, :HH],
                                start=False,
                                stop=True,
                            )
                        src = ps[:].rearrange("p ci tau -> p tau ci")
                        if qkv_i == 0:
                            nc.vector.tensor_copy(
                                qs_sb[:, hsl, cg * CG : cg * CG + CG], src
                            )
                        elif qkv_i == 2:
                            nc.vector.tensor_copy(
                                v_sb[:, hsl, cg * CG : cg * CG + CG], src
                            )
                        else:
                            nc.vector.tensor_copy(
                                k_sb[:, hsl, cg * CG : cg * CG + CG, :],
                                src.unsqueeze(3).broadcast_to([128, HH, CG, 2]),
                            )

        # ---------------- phase 2: criss-cross attention + out-proj ------
        with (
            tc.tile_pool(name="s", bufs=3) as s_pool,
            tc.tile_pool(name="e", bufs=2) as e_pool,
            tc.tile_pool(name="zsm", bufs=6) as z_pool,
            tc.tile_pool(name="ctx", bufs=4) as ctx_pool,
            tc.tile_pool(name="stage", bufs=2) as stage_pool,
            tc.tile_pool(name="ps_z", bufs=2, space="PSUM") as psz,
            tc.tile_pool(name="ps_c", bufs=2, space="PSUM") as psc,
            tc.tile_pool(name="ps_o", bufs=3, space="PSUM") as ps2,
        ):
            stage = [None, None]
            for ch in range(NCH):
                g = ch % flush
                if g == 0:
                    stage = [
                        stage_pool.tile(
                            [128, flush, NSUB, K], F32, tag="st0", name="st0"
                        ),
                        stage_pool.tile(
                            [128, flush, NSUB, K], F32, tag="st1", name="st1"
                        ),
                    ]
                tsl = slice(ch * Tc, (ch + 1) * Tc)

                # s[p, tau, e, (c2,2)] = q[p, tau, (c2,2)] * k[p, tau, e]
                qs4 = (
                    qs_sb[:, tsl, :]
                    .rearrange("p t (c2 two) -> p t c2 two", two=2)
                    .unsqueeze(2)
                    .broadcast_to([128, Tc, C, C // 2, 2])
                )
                k4 = (
                    k_sb[:, tsl, :, :]
                    .unsqueeze(3)
                    .broadcast_to([128, Tc, C, C // 2, 2])
                )
                s_t = s_pool.tile([128, Tc, C, C], BF16)
                s5 = s_t[:].rearrange("p t e (c2 two) -> p t e c2 two", two=2)
                nc.vector.tensor_tensor(s5, qs4, k4, ALU.mult)

                e_t = e_pool.tile([128, Tc, C, C], BF16)
                nc.scalar.activation(e_t[:], s_t[:], AF.Exp)

                # z[p, (tau,c)] = sum_e E via accumulating identity matmuls
                zps = psz.tile([128, Tc * C], F32)
                for e in range(C):
                    nc.tensor.matmul(
                        zps[:],
                        ident[:],
                        e_t[:, :, e, :],
                        start=(e == 0),
                        stop=(e == C - 1),
                    )

                zi_t = z_pool.tile([128, Tc, C], F32, tag="zi")
                nc.vector.reciprocal_approx_fast(
                    zi_t[:], zps[:].rearrange("p (t c) -> p t c", c=C)
                )
                w_t = z_pool.tile([128, Tc, C], BF16, tag="w")
                nc.vector.tensor_tensor(w_t[:], v_sb[:, tsl, :], zi_t[:], ALU.mult)

                # P[p, tau, e, c] = E * w (both packed -> DVE 2x), into s_t
                w4 = w_t[:].unsqueeze(2).broadcast_to([128, Tc, C, C])
                nc.vector.tensor_tensor(s_t[:], e_t[:], w4, ALU.mult)

                # ctx[p, (tau,e)] = sum_c P via accumulating identity matmuls
                cps = psc.tile([128, Tc * C], F32)
                for c in range(C):
                    nc.tensor.matmul(
                        cps[:],
                        ident[:],
                        s_t[:, :, :, c],
                        start=(c == 0),
                        stop=(c == C - 1),
                    )
                ctx_t = ctx_pool.tile([128, Tc, C], BF16, tag="ctx", name="ctx")
                nc.scalar.copy(ctx_t[:], cps[:].rearrange("p (t e) -> p t e", e=C))

                # out-proj: out[(tau4,m), o] = sum_d ctx[(t2,d),(tau,m)] * Wout[d, o]
                # bout folded in via a K=1 accumulating matmul of ones x bout
                for t2 in range(T2):
                    dp = slice(64 * t2, 64 * t2 + 64)
                    po = ps2.tile([128, NSUB, K], F32)
                    for sub in range(NSUB):
                        ssl = slice(sub * 4, sub * 4 + 4)
                        nc.tensor.matmul(
                            po[:, sub, :],
                            ctx_t[dp, ssl, :].rearrange("p t c -> p (t c)"),
                            wo_sb[dp, :],
                            start=True,
                            stop=False,
                        )
                        nc.tensor.matmul(
                            po[:, sub, :],
                            ones_row[0:1, 0:128],
                            bo_sb[0:1, :],
                            start=False,
                            stop=True,
                        )
                    nc.scalar.copy(stage[t2][:, g, :, :], po[:])

                if g == flush - 1:
                    chb = ch // flush
                    ov = out_d[:].rearrange(
                        "(t2 chb chs sub tau) (m o) -> t2 chb tau m chs sub o",
                        t2=T2,
                        chb=NCH // flush,
                        chs=flush,
                        sub=NSUB,
                        tau=4,
                        m=C,
                    )
                    for t2 in range(T2):
                        nc.sync.dma_start(ov[t2, chb], stage[t2][:])

    nc.compile()
    return nc


def _bf16(a):
    import ml_dtypes

    return np.ascontiguousarray(a.astype(ml_dtypes.bfloat16))


def _host_prep(x, Wqkv, bqkv, Wout, bout):
    x = np.ascontiguousarray(np.asarray(x, dtype=np.float32)).reshape(-1, F)
    Wqkv = np.asarray(Wqkv, dtype=np.float32)
    bqkv = np.asarray(bqkv, dtype=np.float32)
    Wout = np.asarray(Wout, dtype=np.float32)
    bout = np.asarray(bout, dtype=np.float32)
    scale = 1.0 / np.sqrt(K)

    # bias rows over the (t2,d) partition dim: [C, 128] with d tiled twice
    def brow(b):  # b: [C, K]
        return _bf16(np.tile(b, (1, 2)))

    common = {
        "wq": _bf16((Wqkv[:, :, :K] * scale).reshape(F, K)),
        "wk": _bf16(Wqkv[:, :, K : 2 * K].reshape(F, K)),
        "wv": _bf16(Wqkv[:, :, 2 * K :].reshape(F, K)),
        "bq": brow(bqkv[:, :K] * scale),
        "bk": brow(bqkv[:, K : 2 * K]),
        "bv": brow(bqkv[:, 2 * K :]),
        "wout": _bf16(np.tile(Wout, (2, 1))),
        "bout": _bf16(np.tile(bout[None, :], (128, 1))),
    }
    return _bf16(x), common


_NC_CACHE = {}


def _get_nc(T):
    if T not in _NC_CACHE:
        _NC_CACHE[T] = build_nc(T=T)
    return _NC_CACHE[T]


def kernel(x, Wqkv, bqkv, Wout, bout, _trace=False):
    from concourse.bass_utils import run_bass_kernel_spmd

    xs, common = _host_prep(x, Wqkv, bqkv, Wout, bout)
    n_tok = xs.shape[0]
    tpc = n_tok // N_CORES
    in_maps = [
        {**common, "x": np.ascontiguousarray(xs[i * tpc : (i + 1) * tpc])}
        for i in range(N_CORES)
    ]
    nc = _get_nc(tpc)
    res = run_bass_kernel_spmd(nc, in_maps, list(range(N_CORES)), trace=_trace)
    out = np.concatenate([res.results[i]["out"] for i in range(N_CORES)], axis=0)
    out = out.reshape(np.asarray(x).shape)
    if _trace:
        kernel.last_results = res
    return out
